# revision 1
# baseline (speedup 1.0000x reference)
"""Cumulative LayerNorm (cLN) Trainium2 Bass kernel.

x: [B=8, C=512, T=16000] fp32.  Per (b, t):
    mean[t] = cumsum_t(sum_c x) / (C*(t+1))
    var[t]  = cumsum_t(sum_c (x - mean[t'])^2) / (C*(t+1))
    out     = (x - mean) / sqrt(var + eps) * gamma + beta

Expansion used on-device (exact in real arithmetic):
    sum_c (x[c,t'] - mean[t'])^2 = ssq[t'] - 2*mean[t']*s1[t'] + C*mean[t']^2

Sharding: data-parallel over batch, one batch per NeuronCore (8 cores).

Per-core pipeline, T processed in 5 chunks of 3200 so x is read from HBM only
once (the chunk stays resident in SBUF between the stats pass and the
normalization pass):
  Stats:   reduce over C via PE matmuls with an all-ones [128,1] stationary
           operand into PSUM rows s1/ssq [1,400];
           squares on ACT; rows evacuated to SBUF and DMA-reshaped into the
           chunk's compact scan layout [128p, 25f] (t_local = p*25 + f).
  Scan:    per-partition prefix sums via DVE tensor_tensor_scan; cross-
           partition carry via a strict-lower-triangular PE matmul; cross-
           chunk carry via a PSUM-accumulated grand total (g) broadcast with a
           second accumulating matmul; pointwise stats; inv = 1/sqrt(var+eps)
           (ACT sqrt + DVE reciprocal); nminv = -mean*inv.
  Norm:    inv/nminv reshaped to [1, 1600] rows (SBUF->SBUF DMA) and
           replicated across all 128 partitions by GPSIMD partition_broadcast
           (no HBM traffic); normalization runs fully in place in the x tiles
           (DVE mul + add, then one ACT affine folding gamma/beta); DMA out.

The ssq reduction matmuls use float32r (full-rate fp32): their input is the
ACT square with a float32r-rounded output, which the BIR verifier requires.
The s1 matmuls consume raw DMA-loaded x and must stay plain fp32 (4 cyc/row).

Built with Bacc (not raw Bass): its compile() pass legalizes multi-wait
instructions into event-semaphore chains — TRN2 hardware instructions can
carry only ONE sync wait.
"""

import numpy as np

B, C, T = 8, 512, 16000
P = 128
NCH = C // P        # 4 chunks of channels
CC = 3200           # t-chunk (must be P * F2 and divide T)
NCC = T // CC       # 5
F2 = CC // P        # 25: compact scan layout free dim per chunk
KB = 400            # PSUM-row block (<=512 fp32, 400 = 16*25)
NKB = CC // KB      # 8 blocks per chunk
HB = 1600           # normalization half-chunk
EPS = 1e-8

_PROG = None


def _build_program():
    from contextlib import ExitStack

    import concourse.bass as bass
    import concourse.tile as tile
    from concourse import bacc, mybir

    f32 = mybir.dt.float32
    f32r = mybir.dt.float32r
    Alu = mybir.AluOpType
    Act = mybir.ActivationFunctionType

    nc = bacc.Bacc("TRN2", debug=False)
    x = nc.dram_tensor("x", [C, T], f32, kind="ExternalInput").ap()
    lstrict = nc.dram_tensor("lstrict", [P, P], f32, kind="ExternalInput").ap()
    recip5 = nc.dram_tensor("recip5", [P, NCC, F2], f32, kind="ExternalInput").ap()
    gamma_pc = nc.dram_tensor("gamma_pc", [P, NCH], f32, kind="ExternalInput").ap()
    beta_pc = nc.dram_tensor("beta_pc", [P, NCH], f32, kind="ExternalInput").ap()
    out = nc.dram_tensor("out", [C, T], f32, kind="ExternalOutput").ap()

    with tile.TileContext(nc) as tc:
        with ExitStack() as ctx:
            singles = ctx.enter_context(tc.tile_pool(name="singles", bufs=1))
            xtp = ctx.enter_context(tc.tile_pool(name="xtp", bufs=11))
            sqp_pool = ctx.enter_context(tc.tile_pool(name="sqp_pool", bufs=4))
            bcp = ctx.enter_context(tc.tile_pool(name="bcp", bufs=3))
            rowp = ctx.enter_context(tc.tile_pool(name="rowp", bufs=4))
            statp = ctx.enter_context(tc.tile_pool(name="statp", bufs=2))
            ps_stat = ctx.enter_context(
                tc.tile_pool(name="ps_stat", bufs=6, space="PSUM")
            )
            ps_carry = ctx.enter_context(
                tc.tile_pool(name="ps_carry", bufs=1, space="PSUM")
            )
            ps_g = ctx.enter_context(tc.tile_pool(name="ps_g", bufs=1, space="PSUM"))

            # ---- constants ----
            ones_col = singles.tile([P, 1], f32)
            nc.vector.memset(ones_col, 1.0)
            ones_row = singles.tile([1, P], f32)
            nc.vector.memset(ones_row, 1.0)
            ones_scan = singles.tile([P, F2], f32)
            nc.vector.memset(ones_scan, 1.0)
            lstrict_sb = singles.tile([P, P], f32)
            nc.sync.dma_start(lstrict_sb, lstrict)
            recip_sb = singles.tile([P, NCC, F2], f32)
            nc.sync.dma_start(recip_sb, recip5)
            gamma_sb = singles.tile([P, NCH], f32)
            nc.sync.dma_start(gamma_sb, gamma_pc)
            beta_sb = singles.tile([P, NCH], f32)
            nc.sync.dma_start(beta_sb, beta_pc)
            eps_sb = singles.tile([P, 1], f32)
            nc.vector.memset(eps_sb, EPS)

            # grand totals over processed chunks: col 0 = sum(s1), col 1 = sum(r)
            g_ps = ps_g.tile([1, 2], f32, tag="g")

            def load_chunk(cc):
                t0 = cc * CC
                xts = []
                for j in range(NCH):
                    xtr = xtp.tile([P, CC], f32r, tag="xt", name=f"xt_{cc}_{j}")
                    nc.sync.dma_start(
                        xtr.bitcast(f32), x[j * P : (j + 1) * P, t0 : t0 + CC]
                    )
                    xts.append(xtr.bitcast(f32))
                return xts

            xts = load_chunk(0)
            for cc in range(NCC):
                t0 = cc * CC

                # ---- stats: channel reductions ----
                s1c = statp.tile([P, F2], f32, tag="s1c", name=f"s1c_{cc}")
                sqc = statp.tile([P, F2], f32, tag="sqc", name=f"sqc_{cc}")
                for kp in range(NKB // 2):
                    xsqs = []
                    for j in range(NCH):
                        xsq = sqp_pool.tile(
                            [P, 2 * KB], f32r, tag="xsq", name=f"xsq_{cc}_{kp}_{j}"
                        )
                        nc.scalar.square(
                            xsq, xts[j][:, kp * 2 * KB : (kp + 1) * 2 * KB]
                        )
                        xsqs.append(xsq)
                    for k2 in range(2):
                        k = kp * 2 + k2
                        s1p = ps_stat.tile([1, KB], f32, tag="st", name=f"s1p_{cc}_{k}")
                        sqp = ps_stat.tile([1, KB], f32, tag="st", name=f"sqp_{cc}_{k}")
                        for j in range(NCH):
                            nc.tensor.matmul(
                                s1p,
                                ones_col,
                                xts[j][:, k * KB : (k + 1) * KB],
                                start=(j == 0),
                                stop=(j == NCH - 1),
                            )
                        for j in range(NCH):
                            nc.tensor.matmul(
                                sqp,
                                ones_col.bitcast(f32r),
                                xsqs[j][:, k2 * KB : (k2 + 1) * KB],
                                start=(j == 0),
                                stop=(j == NCH - 1),
                            )
                        s1row = rowp.tile(
                            [1, KB], f32, tag="rows", name=f"s1r_{cc}_{k}"
                        )
                        nc.vector.tensor_copy(s1row, s1p)
                        sqrow = rowp.tile(
                            [1, KB], f32, tag="rows", name=f"sqr_{cc}_{k}"
                        )
                        nc.scalar.copy(sqrow, sqp)
                        # 400 t's = 16 partitions x 25 in the chunk scan layout
                        nc.sync.dma_start(s1c[16 * k : 16 * k + 16, :], s1row)
                        nc.sync.dma_start(sqc[16 * k : 16 * k + 16, :], sqrow)

                # prefetch the next chunk now: these loads enter the SP
                # DMA queues ahead of this chunk's stores, so they drain
                # during the serial scan chain instead of idling behind it
                xts_next = load_chunk(cc + 1) if cc + 1 < NCC else None

                # ---- scan + pointwise stats (compact [128, 25]) ----
                if cc > 0:
                    g_prev = statp.tile([1, 2], f32, tag="gprev", name=f"gp_{cc}")
                    nc.vector.tensor_copy(g_prev, g_ps)
                cum1 = statp.tile([P, F2], f32, tag="cum1", name=f"cum1_{cc}")
                nc.vector.tensor_tensor_scan(
                    cum1, ones_scan, s1c, 0.0, Alu.mult, Alu.add
                )
                carryb = ps_carry.tile([P, 2], f32, tag="c", name=f"c_{cc}")
                carry1 = carryb[:, 0:1]
                nc.tensor.matmul(
                    carry1,
                    lstrict_sb,
                    cum1[:, F2 - 1 : F2],
                    start=True,
                    stop=(cc == 0),
                )
                if cc > 0:
                    nc.tensor.matmul(
                        carry1,
                        ones_row,
                        g_prev[:, 0:1],
                        start=False,
                        stop=True,
                        skip_group_check=True,
                    )
                nc.tensor.matmul(
                    g_ps[:, 0:1],
                    ones_col,
                    cum1[:, F2 - 1 : F2],
                    start=(cc == 0),
                    stop=(cc == NCC - 1),
                    skip_group_check=True,
                )
                carry1_sb = statp.tile([P, 1], f32, tag="cs1", name=f"cs1_{cc}")
                nc.vector.tensor_copy(carry1_sb, carry1)
                rc = recip_sb[:, cc, :]
                mean_c = statp.tile([P, F2], f32, tag="mean", name=f"mean_{cc}")
                nc.vector.scalar_tensor_tensor(
                    mean_c, cum1, carry1_sb, rc, Alu.add, Alu.mult
                )
                u_c = statp.tile([P, F2], f32, tag="u", name=f"u_{cc}")
                nc.vector.scalar_tensor_tensor(
                    u_c, mean_c, -float(C) / 2.0, s1c, Alu.mult, Alu.add
                )
                v_c = statp.tile([P, F2], f32, tag="v", name=f"v_{cc}")
                nc.vector.tensor_mul(v_c, mean_c, u_c)
                r_c = statp.tile([P, F2], f32, tag="r", name=f"r_{cc}")
                nc.vector.scalar_tensor_tensor(r_c, v_c, -2.0, sqc, Alu.mult, Alu.add)
                cumr = statp.tile([P, F2], f32, tag="cumr", name=f"cumr_{cc}")
                nc.vector.tensor_tensor_scan(
                    cumr, ones_scan, r_c, 0.0, Alu.mult, Alu.add
                )
                carry2 = carryb[:, 1:2]
                nc.tensor.matmul(
                    carry2,
                    lstrict_sb,
                    cumr[:, F2 - 1 : F2],
                    start=True,
                    stop=(cc == 0),
                )
                if cc > 0:
                    nc.tensor.matmul(
                        carry2,
                        ones_row,
                        g_prev[:, 1:2],
                        start=False,
                        stop=True,
                        skip_group_check=True,
                    )
                nc.tensor.matmul(
                    g_ps[:, 1:2],
                    ones_col,
                    cumr[:, F2 - 1 : F2],
                    start=(cc == 0),
                    stop=(cc == NCC - 1),
                    skip_group_check=True,
                )
                carry2_sb = statp.tile([P, 1], f32, tag="cs2", name=f"cs2_{cc}")
                nc.vector.tensor_copy(carry2_sb, carry2)
                var_c = statp.tile([P, F2], f32, tag="var", name=f"var_{cc}")
                nc.vector.scalar_tensor_tensor(
                    var_c, cumr, carry2_sb, rc, Alu.add, Alu.mult
                )
                std_c = statp.tile([P, F2], f32, tag="std", name=f"std_{cc}")
                nc.scalar.activation(std_c, var_c, Act.Sqrt, bias=eps_sb)
                inv_c = statp.tile([P, F2], f32, tag="inv", name=f"inv_{cc}")
                nc.vector.reciprocal(inv_c, std_c)
                nminv_c = statp.tile([P, F2], f32, tag="nminv", name=f"nm_{cc}")
                nc.vector.scalar_tensor_tensor(
                    nminv_c, mean_c, -1.0, inv_c, Alu.mult, Alu.mult
                )
                # ---- normalize (fully in place in the x tiles) ----
                # reshape compact stats into [1, HB] rows (SBUF->SBUF DMA),
                # then replicate across partitions on the idle GPSIMD engine
                PPH = HB // F2
                for h in range(CC // HB):
                    irow = rowp.tile([1, HB], f32, tag="brow", name=f"ir_{cc}_{h}")
                    nc.sync.dma_start(irow, inv_c[h * PPH : (h + 1) * PPH, :])
                    nrow = rowp.tile([1, HB], f32, tag="brow", name=f"nr_{cc}_{h}")
                    nc.sync.dma_start(nrow, nminv_c[h * PPH : (h + 1) * PPH, :])
                    bci = bcp.tile([P, HB], f32, tag="bc", name=f"bci_{cc}_{h}")
                    nc.gpsimd.partition_broadcast(bci, irow)
                    bcm = bcp.tile([P, HB], f32, tag="bc", name=f"bcm_{cc}_{h}")
                    nc.gpsimd.partition_broadcast(bcm, nrow)
                    for j in range(NCH):
                        sl = xts[j][:, h * HB : (h + 1) * HB]
                        nc.vector.tensor_mul(sl, sl, bci)
                        nc.vector.tensor_add(sl, sl, bcm)
                        # per-half affine + store: the first half streams out
                        # while the second half is still multiplying
                        nc.scalar.activation(
                            sl,
                            sl,
                            Act.Identity,
                            bias=beta_sb[:, j : j + 1],
                            scale=gamma_sb[:, j : j + 1],
                        )
                        nc.sync.dma_start(
                            out[j * P : (j + 1) * P, t0 + h * HB : t0 + (h + 1) * HB],
                            sl,
                        )
                xts = xts_next

    nc.finalize()
    return nc


def _make_consts():
    t = np.arange(T, dtype=np.float64).reshape(NCC, P, F2).transpose(1, 0, 2)
    recip5 = np.ascontiguousarray((1.0 / (C * (t + 1.0))).astype(np.float32))
    lstrict = np.triu(np.ones((P, P), dtype=np.float32), k=1)
    return lstrict, recip5


def kernel(x, gamma, beta):
    global _PROG
    from concourse import bass_utils

    x = np.ascontiguousarray(np.asarray(x, dtype=np.float32))
    gamma = np.asarray(gamma, dtype=np.float32).reshape(C)
    beta = np.asarray(beta, dtype=np.float32).reshape(C)

    if _PROG is None:
        _PROG = _build_program()

    lstrict, recip5 = _make_consts()
    gamma_pc = np.ascontiguousarray(gamma.reshape(NCH, P).T)
    beta_pc = np.ascontiguousarray(beta.reshape(NCH, P).T)

    in_maps = [
        {
            "x": np.ascontiguousarray(x[b]),
            "lstrict": lstrict,
            "recip5": recip5,
            "gamma_pc": gamma_pc,
            "beta_pc": beta_pc,
        }
        for b in range(B)
    ]
    res = bass_utils.run_bass_kernel_spmd(_PROG, in_maps, core_ids=list(range(B)))
    return np.stack([res.results[b]["out"] for b in range(B)], axis=0)



# revision 6
# speedup vs baseline: 1.0591x; 1.0591x over previous
"""Cumulative LayerNorm (cLN) Trainium2 Bass kernel.

x: [B=8, C=512, T=16000] fp32.  Per (b, t):
    mean[t] = cumsum_t(sum_c x) / (C*(t+1))
    var[t]  = cumsum_t(sum_c (x - mean[t'])^2) / (C*(t+1))
    out     = (x - mean) / sqrt(var + eps) * gamma + beta

Expansion used on-device (exact in real arithmetic):
    sum_c (x[c,t'] - mean[t'])^2 = ssq[t'] - 2*mean[t']*s1[t'] + C*mean[t']^2

Sharding: data-parallel over batch, one batch per NeuronCore (8 cores).

Per-core pipeline, software-pipelined at chunk granularity (5 chunks of 3200):
while chunk cc is normalized + stored, chunk cc+1 runs stats + scan and
chunk cc+2 streams in.  The host pre-shuffles x to [128, 4, T] (p-major) so
every load/store is ONE big descriptor-friendly DMA per half-chunk.

  Stats:  channel reduction via PE matmuls (f32r, 1 cyc/row) with an all-ones
          stationary column; squares on ACT (f32r out); s1/ssq PSUM rows
          [2, 400] evacuated by single ACT copies into a [2, 3200] row pair,
          then reshaped by one small DMA per stat into the compact per-chunk
          scan layout [128, 25] (t_local = p*25 + f).
  Scan:   per-partition prefix sums via DVE tensor_tensor_scan; cross-
          partition carry via a strict-lower-triangular PE matmul; cross-
          chunk carry kept in SBUF (per-chunk grand total via a tiny PE
          reduction, accumulated with a [1,2] DVE add).
  Norm:   inv/nminv rows DMA-reshaped to [1, 2*1600] and replicated across
          partitions by GPSIMD partition_broadcast; the normalization is two
          DVE passes fully in place in the x tiles:
            pass 1: (x*gamma)*inv      (scalar_tensor_tensor)
            pass 2: (nminv*gamma+beta)+.  (custom-DVE affine_then_add)
          so the gamma/beta affine costs nothing extra.

DMA issue is split across both HWDGE queues: SP carries loads + the small
reshape rows, ACT carries the stores, so a store waiting on the normalize
never head-of-line-blocks the next chunk's stat rows.
"""

import numpy as np

B, C, T = 8, 512, 16000
P = 128
NCH = C // P        # 4 channel groups
CC = 3200           # t-chunk (must be P * F2 and divide T)
NCC = T // CC       # 5
F2 = CC // P        # 25: compact scan layout free dim per chunk
HB = 1600           # half-chunk (load/store/normalize granularity)
KB = 400            # PSUM-row block (fits one 2KB PSUM bank)
NKB_H = HB // KB    # 4 blocks per half
EPS = 1e-8

_PROG = None


def _build_program():
    from contextlib import ExitStack

    import concourse.bass as bass
    import concourse.tile as tile
    from concourse import bacc, mybir

    f32 = mybir.dt.float32
    f32r = mybir.dt.float32r
    Alu = mybir.AluOpType
    Act = mybir.ActivationFunctionType

    nc = bacc.Bacc("TRN2", debug=False)
    xr = nc.dram_tensor("x", [P, NCH, T], f32r, kind="ExternalInput").ap()
    lstrict = nc.dram_tensor("lstrict", [P, P], f32, kind="ExternalInput").ap()
    recip5 = nc.dram_tensor("recip5", [P, NCC, F2], f32, kind="ExternalInput").ap()
    gamma_pc = nc.dram_tensor("gamma_pc", [P, NCH], f32, kind="ExternalInput").ap()
    beta_pc = nc.dram_tensor("beta_pc", [P, NCH], f32, kind="ExternalInput").ap()
    out = nc.dram_tensor("out", [P, NCH, T], f32, kind="ExternalOutput").ap()

    with tile.TileContext(nc) as tc:
        with ExitStack() as ctx:
            singles = ctx.enter_context(tc.tile_pool(name="singles", bufs=1))
            xhp = ctx.enter_context(tc.tile_pool(name="xhp", bufs=5))
            xsqp = ctx.enter_context(tc.tile_pool(name="xsqp", bufs=2))
            srowp = ctx.enter_context(tc.tile_pool(name="srowp", bufs=1))
            s1sqp = ctx.enter_context(tc.tile_pool(name="s1sqp", bufs=2))
            statp = ctx.enter_context(tc.tile_pool(name="statp", bufs=2))
            browp = ctx.enter_context(tc.tile_pool(name="browp", bufs=1))
            bcp = ctx.enter_context(tc.tile_pool(name="bcp", bufs=2))
            ps_stat = ctx.enter_context(
                tc.tile_pool(name="ps_stat", bufs=6, space="PSUM")
            )
            ps_c1 = ctx.enter_context(tc.tile_pool(name="ps_c1", bufs=1, space="PSUM"))
            ps_c2 = ctx.enter_context(tc.tile_pool(name="ps_c2", bufs=1, space="PSUM"))

            # ---- constants ----
            ones_col = singles.tile([P, 1], f32)
            nc.vector.memset(ones_col, 1.0)
            ones_row = singles.tile([1, P], f32)
            nc.vector.memset(ones_row, 1.0)
            ones_scan = singles.tile([P, F2], f32)
            nc.vector.memset(ones_scan, 1.0)
            lstrict_sb = singles.tile([P, P], f32)
            nc.sync.dma_start(lstrict_sb, lstrict)
            recip_sb = singles.tile([P, NCC, F2], f32)
            nc.sync.dma_start(recip_sb, recip5)
            gamma_sb = singles.tile([P, NCH], f32)
            nc.sync.dma_start(gamma_sb, gamma_pc)
            beta_sb = singles.tile([P, NCH], f32)
            nc.sync.dma_start(beta_sb, beta_pc)
            eps_sb = singles.tile([P, 1], f32)
            nc.vector.memset(eps_sb, EPS)
            # running grand totals of (s1, r) over completed chunks, in SBUF
            gtot = singles.tile([1, 2], f32)

            xh = {}  # half index s = 2*cc + hh -> tile [P, NCH, HB]

            def load_half(s):
                cc, hh = divmod(s, 2)
                t0 = cc * CC + hh * HB
                xt = xhp.tile([P, NCH, HB], f32r, tag="xh", name=f"xh_{s}")
                nc.sync.dma_start(xt, xr[:, :, t0 : t0 + HB])
                xh[s] = xt

            def stats(cc):
                """Channel reductions for chunk cc into srow [2, CC]."""
                srow = srowp.tile([2, CC], f32, tag="srow", name=f"srow_{cc}")
                for hh in range(2):
                    xt = xh[2 * cc + hh]
                    xtf = xt.bitcast(f32)
                    for k in range(NKB_H):
                        kc = hh * NKB_H + k
                        ksl = slice(k * KB, (k + 1) * KB)
                        xsq = xsqp.tile(
                            [P, NCH, KB], f32r, tag="xsq", name=f"xsq_{cc}_{kc}"
                        )
                        for j in range(NCH):
                            nc.scalar.square(xsq[:, j, :], xtf[:, j, ksl])
                        s1p = ps_stat.tile([1, KB], f32, tag="st", name=f"s1p_{cc}_{kc}")
                        sqp = ps_stat.tile([1, KB], f32, tag="st", name=f"sqp_{cc}_{kc}")
                        for j in range(NCH):
                            nc.tensor.matmul(
                                s1p,
                                ones_col.bitcast(f32r),
                                xt[:, j, ksl],
                                start=(j == 0),
                                stop=(j == NCH - 1),
                            )
                        for j in range(NCH):
                            nc.tensor.matmul(
                                sqp,
                                ones_col.bitcast(f32r),
                                xsq[:, j, :],
                                start=(j == 0),
                                stop=(j == NCH - 1),
                            )
                        ksl2 = slice(kc * KB, (kc + 1) * KB)
                        nc.scalar.copy(srow[0:1, ksl2], s1p)
                        nc.scalar.copy(srow[1:2, ksl2], sqp)
                s1sq = s1sqp.tile([P, 2, F2], f32, tag="s1sq", name=f"s1sq_{cc}")
                nc.sync.dma_start(s1sq[:, 0, :], srow[0:1, :])
                nc.sync.dma_start(s1sq[:, 1, :], srow[1:2, :])
                return s1sq

            def scan(cc, s1sq):
                """Prefix-scan stats for chunk cc -> invnm [P, 2, F2]."""
                s1c = s1sq[:, 0, :]
                sqc = s1sq[:, 1, :]
                rc = recip_sb[:, cc, :]
                cum1 = statp.tile([P, F2], f32, tag="cum1", name=f"cum1_{cc}")
                nc.vector.tensor_tensor_scan(
                    cum1, ones_scan, s1c, 0.0, Alu.mult, Alu.add
                )
                carry1 = ps_c1.tile([P, 2], f32, tag="c1", name=f"c1_{cc}")
                nc.tensor.matmul(
                    carry1[:, 0:1],
                    lstrict_sb,
                    cum1[:, F2 - 1 : F2],
                    start=True,
                    stop=(cc == 0),
                )
                if cc > 0:
                    nc.tensor.matmul(
                        carry1[:, 0:1],
                        ones_row,
                        gtot[0:1, 0:1],
                        start=False,
                        stop=True,
                        skip_group_check=True,
                    )
                if cc < NCC - 1:
                    nc.tensor.matmul(
                        carry1[0:1, 1:2],
                        ones_col,
                        cum1[:, F2 - 1 : F2],
                        start=True,
                        stop=True,
                        skip_group_check=True,
                    )
                c1_sb = statp.tile([P, 2], f32, tag="cs1", name=f"cs1_{cc}")
                nc.vector.tensor_copy(c1_sb, carry1)
                carry1_sb = c1_sb[:, 0:1]
                mean_c = statp.tile([P, F2], f32, tag="mean", name=f"mean_{cc}")
                nc.vector.scalar_tensor_tensor(
                    mean_c, cum1, carry1_sb, rc, Alu.add, Alu.mult
                )
                u_c = statp.tile([P, F2], f32, tag="u", name=f"u_{cc}")
                nc.vector.scalar_tensor_tensor(
                    u_c, mean_c, -float(C) / 2.0, s1c, Alu.mult, Alu.add
                )
                nc.vector.tensor_mul(u_c, mean_c, u_c)
                nc.vector.scalar_tensor_tensor(u_c, u_c, -2.0, sqc, Alu.mult, Alu.add)
                cumr = statp.tile([P, F2], f32, tag="cumr", name=f"cumr_{cc}")
                nc.vector.tensor_tensor_scan(
                    cumr, ones_scan, u_c, 0.0, Alu.mult, Alu.add
                )
                # fold this chunk's grand total (col 1 of the carry tile) into
                # the SBUF running total AFTER the carry matmul read it
                if cc < NCC - 1:
                    if cc == 0:
                        nc.vector.tensor_copy(gtot[0:1, 0:1], c1_sb[0:1, 1:2])
                    else:
                        nc.vector.tensor_add(
                            gtot[0:1, 0:1], gtot[0:1, 0:1], c1_sb[0:1, 1:2]
                        )
                carry2 = ps_c2.tile([P, 2], f32, tag="c2", name=f"c2_{cc}")
                nc.tensor.matmul(
                    carry2[:, 0:1],
                    lstrict_sb,
                    cumr[:, F2 - 1 : F2],
                    start=True,
                    stop=(cc == 0),
                )
                if cc > 0:
                    nc.tensor.matmul(
                        carry2[:, 0:1],
                        ones_row,
                        gtot[0:1, 1:2],
                        start=False,
                        stop=True,
                        skip_group_check=True,
                    )
                if cc < NCC - 1:
                    nc.tensor.matmul(
                        carry2[0:1, 1:2],
                        ones_col,
                        cumr[:, F2 - 1 : F2],
                        start=True,
                        stop=True,
                        skip_group_check=True,
                    )
                c2_sb = statp.tile([P, 2], f32, tag="cs2", name=f"cs2_{cc}")
                nc.vector.tensor_copy(c2_sb, carry2)
                carry2_sb = c2_sb[:, 0:1]
                var_c = statp.tile([P, F2], f32, tag="var", name=f"var_{cc}")
                nc.vector.scalar_tensor_tensor(
                    var_c, cumr, carry2_sb, rc, Alu.add, Alu.mult
                )
                if cc < NCC - 1:
                    if cc == 0:
                        nc.vector.tensor_copy(gtot[0:1, 1:2], c2_sb[0:1, 1:2])
                    else:
                        nc.vector.tensor_add(
                            gtot[0:1, 1:2], gtot[0:1, 1:2], c2_sb[0:1, 1:2]
                        )
                std_c = statp.tile([P, F2], f32, tag="std", name=f"std_{cc}")
                nc.scalar.activation(std_c, var_c, Act.Sqrt, bias=eps_sb)
                invnm = statp.tile([P, 2, F2], f32, tag="invnm", name=f"invnm_{cc}")
                nc.vector.reciprocal(invnm[:, 0, :], std_c)
                nc.vector.scalar_tensor_tensor(
                    invnm[:, 1, :], mean_c, -1.0, invnm[:, 0, :], Alu.mult, Alu.mult
                )
                return invnm

            def make_bc(cc, hh, invnm):
                """inv/nminv rows for half hh -> broadcast tile [P, 2, HB]."""
                brow = browp.tile([1, 2, HB], f32, tag="brow", name=f"brow_{cc}_{hh}")
                psl = slice(64 * hh, 64 * hh + 64)
                nc.sync.dma_start(brow[:, 0, :], invnm[psl, 0, :])
                nc.sync.dma_start(brow[:, 1, :], invnm[psl, 1, :])
                bc = bcp.tile([P, 2, HB], f32, tag="bc", name=f"bc_{cc}_{hh}")
                nc.gpsimd.partition_broadcast(bc, brow)
                return bc

            def normalize(cc, hh, bc):
                s = 2 * cc + hh
                t0 = cc * CC + hh * HB
                xtf = xh[s].bitcast(f32)
                for j in range(NCH):
                    xj = xtf[:, j, :]
                    nc.vector.scalar_tensor_tensor(
                        xj, xj, gamma_sb[:, j : j + 1], bc[:, 0, :], Alu.mult, Alu.mult
                    )
                    nc.vector.affine_then_add(
                        xj,
                        bc[:, 1, :],
                        xj,
                        scale=gamma_sb[:, j : j + 1],
                        bias=beta_sb[:, j : j + 1],
                    )
                nc.scalar.dma_start(out[:, :, t0 : t0 + HB], xtf)

            # ---- prologue: chunks 0,1 in flight; stats+scan+bc for chunk 0
            for s in range(4):
                load_half(s)
            s1sq0 = stats(0)
            invnm0 = scan(0, s1sq0)
            bcs = [make_bc(0, hh, invnm0) for hh in range(2)]

            # ---- steady-state bodies
            for cc in range(NCC):
                if 2 * (cc + 2) < 2 * NCC:
                    load_half(2 * (cc + 2))
                    load_half(2 * (cc + 2) + 1)
                s1sq_n = stats(cc + 1) if cc + 1 < NCC else None
                for hh in range(2):
                    normalize(cc, hh, bcs[hh])
                if s1sq_n is not None:
                    invnm_n = scan(cc + 1, s1sq_n)
                    bcs = [make_bc(cc + 1, hh, invnm_n) for hh in range(2)]

    nc.finalize()
    return nc


def _make_consts():
    t = np.arange(T, dtype=np.float64).reshape(NCC, P, F2).transpose(1, 0, 2)
    recip5 = np.ascontiguousarray((1.0 / (C * (t + 1.0))).astype(np.float32))
    lstrict = np.triu(np.ones((P, P), dtype=np.float32), k=1)
    return lstrict, recip5


def _make_in_map(xb, gamma, beta):
    """Per-core input dict. xb: [C, T] fp32; gamma/beta: [C]."""
    lstrict, recip5 = _make_consts()
    return {
        "x": np.ascontiguousarray(xb.reshape(NCH, P, T).transpose(1, 0, 2)),
        "lstrict": lstrict,
        "recip5": recip5,
        "gamma_pc": np.ascontiguousarray(gamma.reshape(NCH, P).T),
        "beta_pc": np.ascontiguousarray(beta.reshape(NCH, P).T),
    }


def _from_out_layout(o):
    """Device out [P, NCH, T] -> [C, T]."""
    return np.ascontiguousarray(o.transpose(1, 0, 2).reshape(C, T))


def kernel(x, gamma, beta):
    global _PROG
    from concourse import bass_utils

    x = np.ascontiguousarray(np.asarray(x, dtype=np.float32))
    gamma = np.asarray(gamma, dtype=np.float32).reshape(C)
    beta = np.asarray(beta, dtype=np.float32).reshape(C)

    if _PROG is None:
        _PROG = _build_program()

    in_maps = [_make_in_map(x[b], gamma, beta) for b in range(B)]
    res = bass_utils.run_bass_kernel_spmd(_PROG, in_maps, core_ids=list(range(B)))
    return np.stack(
        [_from_out_layout(res.results[b]["out"]) for b in range(B)], axis=0
    )


# revision 9
# speedup vs baseline: 1.1002x; 1.0388x over previous
"""Cumulative LayerNorm (cLN) Trainium2 Bass kernel.

x: [B=8, C=512, T=16000] fp32.  Per (b, t):
    mean[t] = cumsum_t(sum_c x) / (C*(t+1))
    var[t]  = cumsum_t(sum_c (x - mean[t'])^2) / (C*(t+1))
    out     = (x - mean) / sqrt(var + eps) * gamma + beta

Expansion used on-device (exact in real arithmetic):
    sum_c (x[c,t'] - mean[t'])^2 = ssq[t'] - 2*mean[t']*s1[t'] + C*mean[t']^2

Sharding: data-parallel over batch, one batch per NeuronCore (8 cores).

Per-core pipeline, software-pipelined at chunk granularity (5 chunks of 3200):
while chunk cc is normalized + stored, chunk cc+1 runs stats + scan and
chunk cc+2 streams in.  The host pre-shuffles x to [128, 4, T] (p-major) so
every load/store is ONE big descriptor-friendly DMA per half-chunk.

  Stats:  channel reduction via PE matmuls (f32r, 1 cyc/row) with an all-ones
          stationary column; squares on ACT (f32r out); s1/ssq PSUM rows
          [2, 400] evacuated by single ACT copies into a [2, 3200] row pair,
          then reshaped by one small DMA per stat into the compact per-chunk
          scan layout [128, 25] (t_local = p*25 + f).
  Scan:   per-partition prefix sums via DVE tensor_tensor_scan; cross-
          partition carry via a strict-lower-triangular PE matmul; cross-
          chunk carry kept in SBUF (per-chunk grand total via a tiny PE
          reduction, accumulated with a [1,2] DVE add).
  Norm:   inv/nminv rows DMA-reshaped to [1, 2*1600] and replicated across
          partitions by GPSIMD partition_broadcast; the normalization is two
          DVE passes fully in place in the x tiles:
            pass 1: (x*gamma)*inv      (scalar_tensor_tensor)
            pass 2: (nminv*gamma+beta)+.  (custom-DVE affine_then_add)
          so the gamma/beta affine costs nothing extra.

DMA issue is split across both HWDGE queues: SP carries loads + the small
reshape rows, ACT carries the stores, so a store waiting on the normalize
never head-of-line-blocks the next chunk's stat rows.
"""

import numpy as np

B, C, T = 8, 512, 16000
P = 128
NCH = C // P        # 4 channel groups
CC = 3200           # t-chunk (must be P * F2 and divide T)
NCC = T // CC       # 5
F2 = CC // P        # 25: compact scan layout free dim per chunk
HB = 1600           # half-chunk (load/store/normalize granularity)
KB = 400            # PSUM-row block (fits one 2KB PSUM bank)
NKB_H = HB // KB    # 4 blocks per half
EPS = 1e-8

_PROG = None


def _build_program():
    from contextlib import ExitStack

    import concourse.bass as bass
    import concourse.tile as tile
    from concourse import bacc, mybir

    f32 = mybir.dt.float32
    f32r = mybir.dt.float32r
    Alu = mybir.AluOpType
    Act = mybir.ActivationFunctionType

    nc = bacc.Bacc("TRN2", debug=False)
    xr = nc.dram_tensor("x", [P, NCH, T], f32r, kind="ExternalInput").ap()
    lstrict = nc.dram_tensor("lstrict", [P, P], f32, kind="ExternalInput").ap()
    recip5 = nc.dram_tensor("recip5", [P, NCC, F2], f32, kind="ExternalInput").ap()
    gamma_pc = nc.dram_tensor("gamma_pc", [P, NCH], f32, kind="ExternalInput").ap()
    beta_pc = nc.dram_tensor("beta_pc", [P, NCH], f32, kind="ExternalInput").ap()
    out = nc.dram_tensor("out", [P, NCH, T], f32, kind="ExternalOutput").ap()

    with tile.TileContext(nc) as tc:
        with ExitStack() as ctx:
            singles = ctx.enter_context(tc.tile_pool(name="singles", bufs=1))
            xhp = ctx.enter_context(tc.tile_pool(name="xhp", bufs=5))
            xsqp = ctx.enter_context(tc.tile_pool(name="xsqp", bufs=2))
            srowp = ctx.enter_context(tc.tile_pool(name="srowp", bufs=1))
            s1sqp = ctx.enter_context(tc.tile_pool(name="s1sqp", bufs=2))
            statp = ctx.enter_context(tc.tile_pool(name="statp", bufs=2))
            browp = ctx.enter_context(tc.tile_pool(name="browp", bufs=1))
            bcp = ctx.enter_context(tc.tile_pool(name="bcp", bufs=2))
            ps_stat = ctx.enter_context(
                tc.tile_pool(name="ps_stat", bufs=6, space="PSUM")
            )
            ps_c1 = ctx.enter_context(tc.tile_pool(name="ps_c1", bufs=1, space="PSUM"))
            ps_c2 = ctx.enter_context(tc.tile_pool(name="ps_c2", bufs=1, space="PSUM"))

            # ---- constants ----
            ones_col = singles.tile([P, 1], f32)
            nc.vector.memset(ones_col, 1.0)
            ones_row = singles.tile([1, P], f32)
            nc.vector.memset(ones_row, 1.0)
            ones_scan = singles.tile([P, F2], f32)
            nc.vector.memset(ones_scan, 1.0)
            lstrict_sb = singles.tile([P, P], f32)
            nc.sync.dma_start(lstrict_sb, lstrict)
            recip_sb = singles.tile([P, NCC, F2], f32)
            nc.sync.dma_start(recip_sb, recip5)
            gamma_sb = singles.tile([P, NCH], f32)
            nc.sync.dma_start(gamma_sb, gamma_pc)
            beta_sb = singles.tile([P, NCH], f32)
            nc.sync.dma_start(beta_sb, beta_pc)
            eps_sb = singles.tile([P, 1], f32)
            nc.vector.memset(eps_sb, EPS)
            # running grand totals of (s1, r) over completed chunks, in SBUF
            gtot = singles.tile([1, 2], f32)

            xh = {}  # half index s = 2*cc + hh -> tile [P, NCH, HB]

            def load_half(s):
                cc, hh = divmod(s, 2)
                t0 = cc * CC + hh * HB
                xt = xhp.tile([P, NCH, HB], f32r, tag="xh", name=f"xh_{s}")
                nc.sync.dma_start(xt, xr[:, :, t0 : t0 + HB])
                xh[s] = xt

            def stats(cc):
                """Channel reductions for chunk cc into srow [2, CC]."""
                srow = srowp.tile([33, CC], f32, tag="srow", name=f"srow_{cc}")
                for hh in range(2):
                    xt = xh[2 * cc + hh]
                    xtf = xt.bitcast(f32)
                    for k in range(NKB_H):
                        kc = hh * NKB_H + k
                        ksl = slice(k * KB, (k + 1) * KB)
                        xsq = xsqp.tile(
                            [P, NCH, KB], f32r, tag="xsq", name=f"xsq_{cc}_{kc}"
                        )
                        for j in range(NCH):
                            nc.scalar.square(xsq[:, j, :], xtf[:, j, ksl])
                        s1p = ps_stat.tile([1, KB], f32, tag="st", name=f"s1p_{cc}_{kc}")
                        sqp = ps_stat.tile([1, KB], f32, tag="st", name=f"sqp_{cc}_{kc}")
                        for j in range(NCH):
                            nc.tensor.matmul(
                                s1p,
                                ones_col.bitcast(f32r),
                                xt[:, j, ksl],
                                start=(j == 0),
                                stop=(j == NCH - 1),
                            )
                        for j in range(NCH):
                            nc.tensor.matmul(
                                sqp,
                                ones_col.bitcast(f32r),
                                xsq[:, j, :],
                                start=(j == 0),
                                stop=(j == NCH - 1),
                            )
                        ksl2 = slice(kc * KB, (kc + 1) * KB)
                        nc.scalar.copy(srow[0:1, ksl2], s1p)
                        nc.scalar.copy(srow[32:33, ksl2], sqp)
                s1sq = s1sqp.tile([P, 2, F2], f32, tag="s1sq", name=f"s1sq_{cc}")
                nc.sync.dma_start(s1sq[:, 0, :], srow[0:1, :])
                nc.sync.dma_start(s1sq[:, 1, :], srow[32:33, :])
                return s1sq

            def scan(cc, s1sq):
                """Prefix-scan stats for chunk cc -> invnm [P, 2, F2]."""
                s1c = s1sq[:, 0, :]
                sqc = s1sq[:, 1, :]
                rc = recip_sb[:, cc, :]
                cum1 = statp.tile([P, F2], f32, tag="cum1", name=f"cum1_{cc}")
                nc.vector.tensor_tensor_scan(
                    cum1, ones_scan, s1c, 0.0, Alu.mult, Alu.add
                )
                carry1 = ps_c1.tile([P, 1], f32, tag="c1", name=f"c1_{cc}")
                if cc > 0:
                    nc.tensor.matmul(
                        carry1, ones_row, gtot[0:1, 0:1], start=True, stop=False
                    )
                nc.tensor.matmul(
                    carry1,
                    lstrict_sb,
                    cum1[:, F2 - 1 : F2],
                    start=(cc == 0),
                    stop=True,
                )
                carry1_sb = statp.tile([P, 1], f32, tag="cs1", name=f"cs1_{cc}")
                nc.vector.tensor_copy(carry1_sb, carry1)
                mean_c = statp.tile([P, F2], f32, tag="mean", name=f"mean_{cc}")
                nc.vector.scalar_tensor_tensor(
                    mean_c, cum1, carry1_sb, rc, Alu.add, Alu.mult
                )
                u_c = statp.tile([P, F2], f32, tag="u", name=f"u_{cc}")
                nc.vector.scalar_tensor_tensor(
                    u_c, mean_c, -float(C) / 2.0, s1c, Alu.mult, Alu.add
                )
                nc.vector.tensor_mul(u_c, mean_c, u_c)
                nc.vector.scalar_tensor_tensor(u_c, u_c, -2.0, sqc, Alu.mult, Alu.add)
                cumr = statp.tile([P, F2], f32, tag="cumr", name=f"cumr_{cc}")
                nc.vector.tensor_tensor_scan(
                    cumr, ones_scan, u_c, 0.0, Alu.mult, Alu.add
                )
                carry2 = ps_c2.tile([P, 1], f32, tag="c2", name=f"c2_{cc}")
                if cc > 0:
                    nc.tensor.matmul(
                        carry2, ones_row, gtot[0:1, 1:2], start=True, stop=False
                    )
                nc.tensor.matmul(
                    carry2,
                    lstrict_sb,
                    cumr[:, F2 - 1 : F2],
                    start=(cc == 0),
                    stop=True,
                )
                carry2_sb = statp.tile([P, 1], f32, tag="cs2", name=f"cs2_{cc}")
                nc.vector.tensor_copy(carry2_sb, carry2)
                var_c = statp.tile([P, F2], f32, tag="var", name=f"var_{cc}")
                nc.vector.scalar_tensor_tensor(
                    var_c, cumr, carry2_sb, rc, Alu.add, Alu.mult
                )
                # this chunk's grand totals (s1, r) -> SBUF running total,
                # read by the NEXT chunk's carry matmuls
                if cc < NCC - 1:
                    tot = ps_stat.tile([1, 2], f32, tag="st", name=f"tot_{cc}")
                    nc.tensor.matmul(
                        tot[0:1, 0:1],
                        ones_col,
                        cum1[:, F2 - 1 : F2],
                        start=True,
                        stop=True,
                    )
                    nc.tensor.matmul(
                        tot[0:1, 1:2],
                        ones_col,
                        cumr[:, F2 - 1 : F2],
                        start=True,
                        stop=True,
                    )
                    if cc == 0:
                        nc.vector.tensor_copy(gtot, tot)
                    else:
                        tot_sb = statp.tile([1, 2], f32, tag="tsb", name=f"tsb_{cc}")
                        nc.vector.tensor_copy(tot_sb, tot)
                        nc.vector.tensor_add(gtot, gtot, tot_sb)
                std_c = statp.tile([P, F2], f32, tag="std", name=f"std_{cc}")
                nc.scalar.activation(std_c, var_c, Act.Sqrt, bias=eps_sb)
                invnm = statp.tile([P, 2, F2], f32, tag="invnm", name=f"invnm_{cc}")
                nc.vector.reciprocal(invnm[:, 0, :], std_c)
                nc.vector.scalar_tensor_tensor(
                    invnm[:, 1, :], mean_c, -1.0, invnm[:, 0, :], Alu.mult, Alu.mult
                )
                return invnm

            def make_bc(cc, hh, invnm):
                """inv/nminv rows for half hh -> broadcast tile [P, 2, HB]."""
                brow = browp.tile([1, 2, HB], f32, tag="brow", name=f"brow_{cc}_{hh}")
                psl = slice(64 * hh, 64 * hh + 64)
                nc.sync.dma_start(brow[:, 0, :], invnm[psl, 0, :])
                nc.sync.dma_start(brow[:, 1, :], invnm[psl, 1, :])
                bc = bcp.tile([P, 2, HB], f32, tag="bc", name=f"bc_{cc}_{hh}")
                nc.gpsimd.partition_broadcast(bc, brow)
                return bc

            def normalize(cc, hh, bc):
                s = 2 * cc + hh
                t0 = cc * CC + hh * HB
                xtf = xh[s].bitcast(f32)
                for j in range(NCH):
                    xj = xtf[:, j, :]
                    nc.vector.scalar_tensor_tensor(
                        xj, xj, gamma_sb[:, j : j + 1], bc[:, 0, :], Alu.mult, Alu.mult
                    )
                    nc.vector.affine_then_add(
                        xj,
                        bc[:, 1, :],
                        xj,
                        scale=gamma_sb[:, j : j + 1],
                        bias=beta_sb[:, j : j + 1],
                    )
                nc.scalar.dma_start(out[:, :, t0 : t0 + HB], xtf)

            # ---- prologue: chunks 0,1 in flight; stats+scan+bc for chunk 0
            for s in range(4):
                load_half(s)
            s1sq0 = stats(0)
            invnm0 = scan(0, s1sq0)
            bcs = [make_bc(0, hh, invnm0) for hh in range(2)]

            # ---- steady-state bodies
            for cc in range(NCC):
                if 2 * (cc + 2) < 2 * NCC:
                    load_half(2 * (cc + 2))
                    load_half(2 * (cc + 2) + 1)
                s1sq_n = stats(cc + 1) if cc + 1 < NCC else None
                for hh in range(2):
                    normalize(cc, hh, bcs[hh])
                if s1sq_n is not None:
                    invnm_n = scan(cc + 1, s1sq_n)
                    bcs = [make_bc(cc + 1, hh, invnm_n) for hh in range(2)]

    nc.finalize()
    return nc


def _make_consts():
    t = np.arange(T, dtype=np.float64).reshape(NCC, P, F2).transpose(1, 0, 2)
    recip5 = np.ascontiguousarray((1.0 / (C * (t + 1.0))).astype(np.float32))
    lstrict = np.triu(np.ones((P, P), dtype=np.float32), k=1)
    return lstrict, recip5


def _make_in_map(xb, gamma, beta):
    """Per-core input dict. xb: [C, T] fp32; gamma/beta: [C]."""
    lstrict, recip5 = _make_consts()
    return {
        "x": np.ascontiguousarray(xb.reshape(NCH, P, T).transpose(1, 0, 2)),
        "lstrict": lstrict,
        "recip5": recip5,
        "gamma_pc": np.ascontiguousarray(gamma.reshape(NCH, P).T),
        "beta_pc": np.ascontiguousarray(beta.reshape(NCH, P).T),
    }


def _from_out_layout(o):
    """Device out [P, NCH, T] -> [C, T]."""
    return np.ascontiguousarray(o.transpose(1, 0, 2).reshape(C, T))


def kernel(x, gamma, beta):
    global _PROG
    from concourse import bass_utils

    x = np.ascontiguousarray(np.asarray(x, dtype=np.float32))
    gamma = np.asarray(gamma, dtype=np.float32).reshape(C)
    beta = np.asarray(beta, dtype=np.float32).reshape(C)

    if _PROG is None:
        _PROG = _build_program()

    in_maps = [_make_in_map(x[b], gamma, beta) for b in range(B)]
    res = bass_utils.run_bass_kernel_spmd(_PROG, in_maps, core_ids=list(range(B)))
    return np.stack(
        [_from_out_layout(res.results[b]["out"]) for b in range(B)], axis=0
    )


# revision 10
# speedup vs baseline: 1.1497x; 1.0450x over previous
"""Cumulative LayerNorm (cLN) Trainium2 Bass kernel.

x: [B=8, C=512, T=16000] fp32.  Per (b, t):
    mean[t] = cumsum_t(sum_c x) / (C*(t+1))
    var[t]  = cumsum_t(sum_c (x - mean[t'])^2) / (C*(t+1))
    out     = (x - mean) / sqrt(var + eps) * gamma + beta

Expansion used on-device (exact in real arithmetic):
    sum_c (x[c,t'] - mean[t'])^2 = ssq[t'] - 2*mean[t']*s1[t'] + C*mean[t']^2

Sharding: data-parallel over batch, one batch per NeuronCore (8 cores).

Per-core pipeline, software-pipelined at chunk granularity (5 chunks of 3200):
while chunk cc is normalized + stored, chunk cc+1 runs stats + scan and
chunk cc+2 streams in.  The host pre-shuffles x to [128, 4, T] (p-major) so
every load/store is ONE big descriptor-friendly DMA per half-chunk.

  Stats:  channel reduction via PE matmuls (f32r, 1 cyc/row) with an all-ones
          stationary column; squares on ACT (f32r out); s1/ssq PSUM rows
          [2, 400] evacuated by single ACT copies into a [2, 3200] row pair,
          then reshaped by one small DMA per stat into the compact per-chunk
          scan layout [128, 25] (t_local = p*25 + f).
  Scan:   per-partition prefix sums via DVE tensor_tensor_scan; cross-
          partition carry via a strict-lower-triangular PE matmul; cross-
          chunk carry kept in SBUF (per-chunk grand total via a tiny PE
          reduction, accumulated with a [1,2] DVE add).
  Norm:   inv/nminv rows DMA-reshaped to [1, 2*1600] and replicated across
          partitions by GPSIMD partition_broadcast; the normalization is two
          DVE passes fully in place in the x tiles:
            pass 1: (x*gamma)*inv      (scalar_tensor_tensor)
            pass 2: (nminv*gamma+beta)+.  (custom-DVE affine_then_add)
          so the gamma/beta affine costs nothing extra.

DMA issue is split across both HWDGE queues: SP carries loads + the small
reshape rows, ACT carries the stores, so a store waiting on the normalize
never head-of-line-blocks the next chunk's stat rows.
"""

import numpy as np

B, C, T = 8, 512, 16000
P = 128
NCH = C // P        # 4 channel groups
CC = 3200           # t-chunk (must be P * F2 and divide T)
NCC = T // CC       # 5
F2 = CC // P        # 25: compact scan layout free dim per chunk
HB = 1600           # half-chunk (load/store/normalize granularity)
KB = 400            # PSUM-row block (fits one 2KB PSUM bank)
NKB_H = HB // KB    # 4 blocks per half
EPS = 1e-8

_PROG = None


def _build_program():
    from contextlib import ExitStack

    import concourse.bass as bass
    import concourse.tile as tile
    from concourse import bacc, mybir

    f32 = mybir.dt.float32
    f32r = mybir.dt.float32r
    Alu = mybir.AluOpType
    Act = mybir.ActivationFunctionType

    nc = bacc.Bacc("TRN2", debug=False)
    xr = nc.dram_tensor("x", [P, NCH, T], f32r, kind="ExternalInput").ap()
    lstrict = nc.dram_tensor("lstrict", [P, P], f32, kind="ExternalInput").ap()
    recip5 = nc.dram_tensor("recip5", [P, NCC, F2], f32, kind="ExternalInput").ap()
    gamma_pc = nc.dram_tensor("gamma_pc", [P, NCH], f32, kind="ExternalInput").ap()
    beta_pc = nc.dram_tensor("beta_pc", [P, NCH], f32, kind="ExternalInput").ap()
    out = nc.dram_tensor("out", [P, NCH, T], f32, kind="ExternalOutput").ap()

    with tile.TileContext(nc) as tc:
        with ExitStack() as ctx:
            singles = ctx.enter_context(tc.tile_pool(name="singles", bufs=1))
            xhp = ctx.enter_context(tc.tile_pool(name="xhp", bufs=5))
            xsqp = ctx.enter_context(tc.tile_pool(name="xsqp", bufs=2))
            srowp = ctx.enter_context(tc.tile_pool(name="srowp", bufs=1))
            s1sqp = ctx.enter_context(tc.tile_pool(name="s1sqp", bufs=2))
            statp = ctx.enter_context(tc.tile_pool(name="statp", bufs=2))
            browp = ctx.enter_context(tc.tile_pool(name="browp", bufs=1))
            bcp = ctx.enter_context(tc.tile_pool(name="bcp", bufs=2))
            ps_stat = ctx.enter_context(
                tc.tile_pool(name="ps_stat", bufs=6, space="PSUM")
            )
            ps_c1 = ctx.enter_context(tc.tile_pool(name="ps_c1", bufs=1, space="PSUM"))
            ps_c2 = ctx.enter_context(tc.tile_pool(name="ps_c2", bufs=1, space="PSUM"))

            # ---- constants ----
            ones_col = singles.tile([P, 1], f32)
            nc.vector.memset(ones_col, 1.0)
            ones_row = singles.tile([1, P], f32)
            nc.vector.memset(ones_row, 1.0)
            ones_scan = singles.tile([P, F2], f32)
            nc.vector.memset(ones_scan, 1.0)
            lstrict_sb = singles.tile([P, P], f32)
            nc.sync.dma_start(lstrict_sb, lstrict)
            recip_sb = singles.tile([P, NCC, F2], f32)
            nc.sync.dma_start(recip_sb, recip5)
            gamma_sb = singles.tile([P, NCH], f32)
            nc.sync.dma_start(gamma_sb, gamma_pc)
            beta_sb = singles.tile([P, NCH], f32)
            nc.sync.dma_start(beta_sb, beta_pc)
            eps_sb = singles.tile([P, 1], f32)
            nc.vector.memset(eps_sb, EPS)
            # running grand totals of (s1, r) over completed chunks, in SBUF
            gtot = singles.tile([1, 2], f32)

            xh = {}  # half index s = 2*cc + hh -> tile [P, NCH, HB]

            def load_half(s):
                cc, hh = divmod(s, 2)
                t0 = cc * CC + hh * HB
                xt = xhp.tile([P, NCH, HB], f32r, tag="xh", name=f"xh_{s}")
                nc.sync.dma_start(xt, xr[:, :, t0 : t0 + HB])
                xh[s] = xt

            def stats(cc):
                """Channel reductions for chunk cc.  Emits ACT squares, PE
                matmuls and ACT sq-row evacs inline; returns the DVE s1-row
                evac closures + the reshape-DMA closure for interleaving, and
                the scan-input tile."""
                srow = srowp.tile([33, CC], f32, tag="srow", name=f"srow_{cc}")
                s1sq = s1sqp.tile([P, 2, F2], f32, tag="s1sq", name=f"s1sq_{cc}")
                s1_evacs = []
                for hh in range(2):
                    xt = xh[2 * cc + hh]
                    xtf = xt.bitcast(f32)
                    for kp in range(2):
                        xsq = xsqp.tile(
                            [P, NCH, 2 * KB], f32r, tag="xsq", name=f"xsq_{cc}_{hh}_{kp}"
                        )
                        psl = slice(kp * 2 * KB, (kp + 1) * 2 * KB)
                        for j in range(NCH):
                            nc.scalar.square(xsq[:, j, :], xtf[:, j, psl])
                        for k2 in range(2):
                            k = kp * 2 + k2
                            kc = hh * NKB_H + k
                            ksl = slice(k * KB, (k + 1) * KB)
                            s1p = ps_stat.tile(
                                [1, KB], f32, tag="st", name=f"s1p_{cc}_{kc}"
                            )
                            sqp = ps_stat.tile(
                                [1, KB], f32, tag="st", name=f"sqp_{cc}_{kc}"
                            )
                            for j in range(NCH):
                                nc.tensor.matmul(
                                    s1p,
                                    ones_col.bitcast(f32r),
                                    xt[:, j, ksl],
                                    start=(j == 0),
                                    stop=(j == NCH - 1),
                                )
                            for j in range(NCH):
                                nc.tensor.matmul(
                                    sqp,
                                    ones_col.bitcast(f32r),
                                    xsq[:, j, k2 * KB : (k2 + 1) * KB],
                                    start=(j == 0),
                                    stop=(j == NCH - 1),
                                )
                            ksl2 = slice(kc * KB, (kc + 1) * KB)
                            nc.scalar.copy(srow[32:33, ksl2], sqp)
                            s1_evacs.append(
                                lambda ksl2=ksl2, s1p=s1p: nc.vector.tensor_copy(
                                    srow[0:1, ksl2], s1p
                                )
                            )

                def reshape():
                    nc.sync.dma_start(s1sq[:, 0, :], srow[0:1, :])
                    nc.sync.dma_start(s1sq[:, 1, :], srow[32:33, :])

                return s1_evacs, reshape, s1sq

            def scan_steps(cc, s1sq):
                """Prefix-scan stats for chunk cc as a list of step closures
                (interleaved between normalize ops by the caller).
                Returns (steps, invnm tile)."""
                s1c = s1sq[:, 0, :]
                sqc = s1sq[:, 1, :]
                rc = recip_sb[:, cc, :]
                cum1 = statp.tile([P, F2], f32, tag="cum1", name=f"cum1_{cc}")
                carry1 = ps_c1.tile([P, 1], f32, tag="c1", name=f"c1_{cc}")
                carry1_sb = statp.tile([P, 1], f32, tag="cs1", name=f"cs1_{cc}")
                mean_c = statp.tile([P, F2], f32, tag="mean", name=f"mean_{cc}")
                u_c = statp.tile([P, F2], f32, tag="u", name=f"u_{cc}")
                cumr = statp.tile([P, F2], f32, tag="cumr", name=f"cumr_{cc}")
                carry2 = ps_c2.tile([P, 1], f32, tag="c2", name=f"c2_{cc}")
                carry2_sb = statp.tile([P, 1], f32, tag="cs2", name=f"cs2_{cc}")
                var_c = statp.tile([P, F2], f32, tag="var", name=f"var_{cc}")
                std_c = statp.tile([P, F2], f32, tag="std", name=f"std_{cc}")
                invnm = statp.tile([P, 2, F2], f32, tag="invnm", name=f"invnm_{cc}")
                last = cc == NCC - 1
                tot = (
                    None
                    if last
                    else ps_stat.tile([1, 2], f32, tag="st", name=f"tot_{cc}")
                )

                def s0():
                    nc.vector.tensor_tensor_scan(
                        cum1, ones_scan, s1c, 0.0, Alu.mult, Alu.add
                    )
                    if cc > 0:
                        nc.tensor.matmul(
                            carry1, ones_row, gtot[0:1, 0:1], start=True, stop=False
                        )
                    nc.tensor.matmul(
                        carry1,
                        lstrict_sb,
                        cum1[:, F2 - 1 : F2],
                        start=(cc == 0),
                        stop=True,
                    )
                    if not last:
                        nc.tensor.matmul(
                            tot[0:1, 0:1],
                            ones_col,
                            cum1[:, F2 - 1 : F2],
                            start=True,
                            stop=True,
                        )

                def s1():
                    nc.vector.tensor_copy(carry1_sb, carry1)

                def s2():
                    nc.vector.scalar_tensor_tensor(
                        mean_c, cum1, carry1_sb, rc, Alu.add, Alu.mult
                    )

                def s3():
                    nc.vector.scalar_tensor_tensor(
                        u_c, mean_c, -float(C) / 2.0, s1c, Alu.mult, Alu.add
                    )
                    nc.vector.tensor_mul(u_c, mean_c, u_c)

                def s4():
                    nc.vector.scalar_tensor_tensor(
                        u_c, u_c, -2.0, sqc, Alu.mult, Alu.add
                    )

                def s5():
                    nc.vector.tensor_tensor_scan(
                        cumr, ones_scan, u_c, 0.0, Alu.mult, Alu.add
                    )
                    if cc > 0:
                        nc.tensor.matmul(
                            carry2, ones_row, gtot[0:1, 1:2], start=True, stop=False
                        )
                    nc.tensor.matmul(
                        carry2,
                        lstrict_sb,
                        cumr[:, F2 - 1 : F2],
                        start=(cc == 0),
                        stop=True,
                    )
                    if not last:
                        nc.tensor.matmul(
                            tot[0:1, 1:2],
                            ones_col,
                            cumr[:, F2 - 1 : F2],
                            start=True,
                            stop=True,
                        )

                def s6():
                    nc.vector.tensor_copy(carry2_sb, carry2)

                def s7():
                    nc.vector.scalar_tensor_tensor(
                        var_c, cumr, carry2_sb, rc, Alu.add, Alu.mult
                    )
                    nc.scalar.activation(std_c, var_c, Act.Sqrt, bias=eps_sb)

                def s8():
                    if last:
                        return
                    if cc == 0:
                        nc.vector.tensor_copy(gtot, tot)
                    else:
                        tot_sb = statp.tile([1, 2], f32, tag="tsb", name=f"tsb_{cc}")
                        nc.vector.tensor_copy(tot_sb, tot)
                        nc.vector.tensor_add(gtot, gtot, tot_sb)

                def s9():
                    nc.vector.reciprocal(invnm[:, 0, :], std_c)

                def s10():
                    nc.vector.scalar_tensor_tensor(
                        invnm[:, 1, :], mean_c, -1.0, invnm[:, 0, :], Alu.mult, Alu.mult
                    )

                return [s0, s1, s2, s3, s4, s5, s6, s7, s8, s9, s10], invnm

            def make_bc(cc, hh, invnm):
                """inv/nminv rows for half hh -> broadcast tile [P, 2, HB]."""
                brow = browp.tile([1, 2, HB], f32, tag="brow", name=f"brow_{cc}_{hh}")
                psl = slice(64 * hh, 64 * hh + 64)
                nc.sync.dma_start(brow[:, 0, :], invnm[psl, 0, :])
                nc.sync.dma_start(brow[:, 1, :], invnm[psl, 1, :])
                bc = bcp.tile([P, 2, HB], f32, tag="bc", name=f"bc_{cc}_{hh}")
                nc.gpsimd.partition_broadcast(bc, brow)
                return bc

            def norm_ops(cc, bcs):
                """16 normalize closures for chunk cc (h0 then h1, mul/add
                pairs per channel group), in place in the x tiles."""
                ops = []
                for hh in range(2):
                    xtf = xh[2 * cc + hh].bitcast(f32)
                    bc = bcs[hh]
                    for j in range(NCH):
                        xj = xtf[:, j, :]

                        def mul(xj=xj, bc=bc, j=j):
                            nc.vector.scalar_tensor_tensor(
                                xj,
                                xj,
                                gamma_sb[:, j : j + 1],
                                bc[:, 0, :],
                                Alu.mult,
                                Alu.mult,
                            )

                        def add(xj=xj, bc=bc, j=j):
                            nc.vector.affine_then_add(
                                xj,
                                bc[:, 1, :],
                                xj,
                                scale=gamma_sb[:, j : j + 1],
                                bias=beta_sb[:, j : j + 1],
                            )

                        ops.append(mul)
                        ops.append(add)
                return ops

            def store(cc, hh):
                t0 = cc * CC + hh * HB
                xtf = xh[2 * cc + hh].bitcast(f32)
                nc.sync.dma_start(out[:, :, t0 : t0 + HB], xtf)

            # ---- prologue: chunks 0,1 and half of 2 in flight; full
            # stats+scan+bc chain for chunk 0 (nothing to overlap with yet)
            for s in range(5):
                load_half(s)
            ev0, rs0, s1sq0 = stats(0)
            for e in ev0:
                e()
            rs0()
            steps0, invnm0 = scan_steps(0, s1sq0)
            for st in steps0:
                st()
            bcs = [make_bc(0, hh, invnm0) for hh in range(2)]

            # ---- software-pipelined bodies: normalize/store chunk cc while
            # chunk cc+1 runs stats+scan and chunk cc+2 streams in
            for cc in range(NCC):
                for s in (5 + 2 * cc, 6 + 2 * cc):
                    if s < 2 * NCC:
                        load_half(s)
                N = norm_ops(cc, bcs)
                if cc + 1 < NCC:
                    evacs, reshape, s1sq_n = stats(cc + 1)
                    S, invnm_n = scan_steps(cc + 1, s1sq_n)
                    # interleave: s1 evacs ride the first normalize ops, the
                    # scan chain rides the rest so its serial latency hides
                    # under normalize throughput
                    N[0]()
                    evacs[0](); evacs[1]()
                    N[1]()
                    evacs[2](); evacs[3]()
                    N[2]()
                    evacs[4](); evacs[5]()
                    N[3]()
                    evacs[6](); evacs[7]()
                    reshape()
                    N[4](); N[5](); N[6](); N[7]()
                    store(cc, 0)
                    S[0]()
                    N[8]()
                    S[1](); S[2]()
                    N[9]()
                    S[3](); S[4]()
                    N[10]()
                    S[5]()
                    N[11]()
                    S[6](); S[7]()
                    N[12]()
                    S[8]()
                    N[13]()
                    S[9](); S[10]()
                    bc0 = make_bc(cc + 1, 0, invnm_n)
                    N[14]()
                    bc1 = make_bc(cc + 1, 1, invnm_n)
                    N[15]()
                    store(cc, 1)
                    bcs = [bc0, bc1]
                else:
                    for i in range(8):
                        N[i]()
                    store(cc, 0)
                    for i in range(8, 16):
                        N[i]()
                    store(cc, 1)

    nc.finalize()
    return nc


def _make_consts():
    t = np.arange(T, dtype=np.float64).reshape(NCC, P, F2).transpose(1, 0, 2)
    recip5 = np.ascontiguousarray((1.0 / (C * (t + 1.0))).astype(np.float32))
    lstrict = np.triu(np.ones((P, P), dtype=np.float32), k=1)
    return lstrict, recip5


def _make_in_map(xb, gamma, beta):
    """Per-core input dict. xb: [C, T] fp32; gamma/beta: [C]."""
    lstrict, recip5 = _make_consts()
    return {
        "x": np.ascontiguousarray(xb.reshape(NCH, P, T).transpose(1, 0, 2)),
        "lstrict": lstrict,
        "recip5": recip5,
        "gamma_pc": np.ascontiguousarray(gamma.reshape(NCH, P).T),
        "beta_pc": np.ascontiguousarray(beta.reshape(NCH, P).T),
    }


def _from_out_layout(o):
    """Device out [P, NCH, T] -> [C, T]."""
    return np.ascontiguousarray(o.transpose(1, 0, 2).reshape(C, T))


def kernel(x, gamma, beta):
    global _PROG
    from concourse import bass_utils

    x = np.ascontiguousarray(np.asarray(x, dtype=np.float32))
    gamma = np.asarray(gamma, dtype=np.float32).reshape(C)
    beta = np.asarray(beta, dtype=np.float32).reshape(C)

    if _PROG is None:
        _PROG = _build_program()

    in_maps = [_make_in_map(x[b], gamma, beta) for b in range(B)]
    res = bass_utils.run_bass_kernel_spmd(_PROG, in_maps, core_ids=list(range(B)))
    return np.stack(
        [_from_out_layout(res.results[b]["out"]) for b in range(B)], axis=0
    )
